# revision 21
# baseline (speedup 1.0000x reference)
"""BCM_Conv2d_fft kernel for Trainium2 (8 NeuronCores, batch-parallel).

The reference is a block-circulant 3x3 conv computed via per-block
rfft/irfft over the channel-block axis (block size 8). Per-frequency the
block products are independent, so in a real-DFT channel basis the
256->256 channel mixing matrix of each conv tap is block-diagonal with
frequency groups {f0:32, f4:32, f1:64} -> chunk0 and {f2:64, f3:64} ->
chunk1: the conv needs 9 matmuls per output tile per chunk (18 total),
which meets the K-streaming lower bound (9 positions x 256 components /
128 K-rows per pass).

The DFT (fwd) and inverse (inv) stages exploit a finer structure: each
channel block's components come only from its own 8 channels, so the
A / Ainv 128x128 chunk blocks have only 64 useful rows / cols. With a
partition layout that groups blocks 0-15 and 16-31 into 64-partition
halves (chunk1 flipped), fwd runs as 2 passes (one per input chunk,
producing halves of BOTH xhat chunks) and inv as 2 passes (one per
output chunk, consuming assembled P/Q tiles) - all inter-stage
PSUM->SBUF copies stay partition-aligned. Per output tile: 2 fwd + 18
conv + 2 inv passes (was 4 + 18 + 4).

Everything streams in bf16 (PSUM accumulates f32): same PE rate as
float32r at these tile sizes, but half the HBM traffic for x, weights
and out, and half-rate LDWEIGHTS via FWL. Max rel err ~3e-3 (tol 2e-2).

Sharding: batch B=8 -> one image per core.
"""

import os

import ml_dtypes
import numpy as np

import concourse.bacc as bacc
import concourse.mybir as mybir
import concourse.tile as tile
from concourse.bass import ts
from concourse.bass_utils import run_bass_kernel_spmd

N_CORES = 8
C = 256
H = W = 56
HP = H + 2
KK = 3
BS = 8
L = H * W
RPT = 8                  # output rows per tile
NT = RPT * W             # 448 pixels per tile
N_TILES = L // NT        # 7
MCH = C // 128           # 2 channel chunks

F32 = mybir.dt.float32
BF16 = mybir.dt.bfloat16
NP_BF16 = ml_dtypes.bfloat16

# weight block column indices in the packed wts tensor [128, 22*128]
FWD_BLK = lambda i: i                    # i = input chunk
CONV_BLK = lambda pos, c: 2 + pos * MCH + c
INV_BLK = lambda m: 20 + m               # m = output chunk
N_BLKS = 22

# real-DFT components per chunk: chunk0 = {f0, f1re, f1im, f4},
# chunk1 = {f2re, f2im, f3re, f3im} (closed under conv's re/im mixing)
C0 = [0, 1, 2, 7]
C1 = [3, 4, 5, 6]

LAST_RESULT = None


def _pc(c, bk, j):
    """Partition of (block bk, comp-index j) within xhat chunk c.

    chunk0: blocks 0-15 at parts 0-63; chunk1: blocks 16-31 at parts
    0-63 (flipped so all fwd/inv half-copies are partition-aligned).
    """
    if c == 0:
        return (bk % 16) * 4 + j + 64 * (bk // 16)
    return (bk % 16) * 4 + j + 64 * (1 - bk // 16)


def _pack_weights(w: np.ndarray) -> np.ndarray:
    """-> [128, 22*128] bf16: lhsT blocks for fwd(2), conv(18), inv(2)."""
    F = np.zeros((8, 8))
    FI = np.fft.rfft(np.eye(8), axis=-1)
    F[0] = FI[:, 0].real
    F[1], F[2] = FI[:, 1].real, FI[:, 1].imag
    F[3], F[4] = FI[:, 2].real, FI[:, 2].imag
    F[5], F[6] = FI[:, 3].real, FI[:, 3].imag
    F[7] = FI[:, 4].real
    Finv = np.linalg.inv(F)
    wf = np.fft.rfft(w.astype(np.float64), axis=-1)  # [32, 288, 5]

    wts = np.zeros((128, N_BLKS * 128), np.float64)

    def put(idx, lhsT):
        wts[:, idx * 128:(idx + 1) * 128] = lhsT

    # fwd pass i (K = x chunk i = blocks 16i..16i+15): M low half feeds
    # chunk i's parts 0-63, high half feeds the other chunk's parts
    # 64-127 (both hold blocks 16i..16i+15 by the _pc layout).
    for i in range(MCH):
        Lk = np.zeros((128, 128))
        own, other = (C0, C1) if i == 0 else (C1, C0)
        for bkl in range(16):
            for j, comp in enumerate(own):
                Lk[bkl * 8:(bkl + 1) * 8, bkl * 4 + j] = F[comp]
            for j, comp in enumerate(other):
                Lk[bkl * 8:(bkl + 1) * 8, 64 + bkl * 4 + j] = F[comp]
        put(FWD_BLK(i), Lk)

    for pos in range(9):
        for c in range(MCH):
            M = np.zeros((128, 128))
            for pb in range(32):
                rp = lambda j: _pc(c, pb, j)
                for kb in range(32):
                    cp = lambda j: _pc(c, kb, j)
                    Wc = wf[pb, pos * 32 + kb, :]
                    if c == 0:
                        M[rp(0), cp(0)] += Wc[0].real            # f0
                        M[rp(3), cp(3)] += Wc[4].real            # f4
                        Wr, Wi = Wc[1].real, Wc[1].imag          # f1
                        M[rp(1), cp(1)] += Wr
                        M[rp(1), cp(2)] += -Wi
                        M[rp(2), cp(1)] += Wi
                        M[rp(2), cp(2)] += Wr
                    else:
                        Wr, Wi = Wc[2].real, Wc[2].imag          # f2
                        M[rp(0), cp(0)] += Wr
                        M[rp(0), cp(1)] += -Wi
                        M[rp(1), cp(0)] += Wi
                        M[rp(1), cp(1)] += Wr
                        Wr, Wi = Wc[3].real, Wc[3].imag          # f3
                        M[rp(2), cp(2)] += Wr
                        M[rp(2), cp(3)] += -Wi
                        M[rp(3), cp(2)] += Wi
                        M[rp(3), cp(3)] += Wr
            put(CONV_BLK(pos, c), M.T)

    # inv pass m consumes P (m=0) / Q (m=1): parts 0-63 hold this out
    # chunk's blocks from its own-chunk conv psum, 64-127 from the other
    for mc in range(MCH):
        Lk = np.zeros((128, 128))
        for k in range(128):
            half, kk = k // 64, k % 64
            bkl, j = kk // 4, kk % 4
            comp = (C0 if (half == 0) == (mc == 0) else C1)[j]
            for e in range(8):
                Lk[k, bkl * 8 + e] = Finv[e, comp]
        put(INV_BLK(mc), Lk)
    return wts.astype(NP_BF16)


def _kernel_body(tc, x, wts, bias, out):
    nc = tc.nc
    ident = mybir.ActivationFunctionType.Identity
    with (
        tc.tile_pool(name="const", bufs=1) as const_pool,
        tc.tile_pool(name="xp", bufs=1) as xp_pool,
        tc.tile_pool(name="xh", bufs=1) as xh_pool,
        tc.tile_pool(name="oh", bufs=6) as oh_pool,
        tc.tile_pool(name="ob", bufs=4) as ob_pool,
        tc.tile_pool(name="psf", bufs=3, space="PSUM") as psf_pool,
        tc.tile_pool(name="psc", bufs=3, space="PSUM") as psc_pool,
        tc.tile_pool(name="psi", bufs=2, space="PSUM") as psi_pool,
    ):
        # DMA issue order is first-needed-first: fwd weights, then the
        # first input rows; conv weights stream on the ACT queue in
        # parallel with the remaining x rows on sync/gpsimd queues.
        wt_sb = const_pool.tile([128, N_BLKS * 128], BF16)
        blk = lambda idx: wt_sb[:, ts(idx, 128)]

        # dummy operand for PE warm-up matmuls (content irrelevant)
        dummy = const_pool.tile([128, NT], BF16)
        nc.gpsimd.memset(dummy[:], 0.0)

        # x arrives unpadded [C, H, W] and stays unpadded in SBUF so the
        # DMA moves large contiguous per-partition spans; the zero
        # padding is materialized in xhat by the fwd drain instead.
        xq = []
        for i in range(MCH):
            xq_t = xp_pool.tile([128, H * W], BF16, tag=f"xp{i}")
            xq.append(xq_t)
        row_splits = [0, 8, 16, 24, 32, 44, H]
        eng = [nc.sync, nc.gpsimd]
        for r0, r1 in zip(row_splits[:-1], row_splits[1:]):
            for i in range(MCH):
                eng[i].dma_start(
                    out=xq[i][:, r0 * W:r1 * W],
                    in_=x[ts(i, 128), r0:r1, :].rearrange("p h w -> p (h w)"),
                )
            if r0 == 0:
                # fwd weights lead the ACT queue so the first matmul is
                # gated only by the first x rows; conv weights follow
                nc.scalar.dma_start(out=wt_sb[:, 0:2 * 128],
                                    in_=wts[:, 0:2 * 128])
                nc.scalar.dma_start(out=wt_sb[:, 2 * 128:11 * 128],
                                    in_=wts[:, 2 * 128:11 * 128])
                nc.scalar.dma_start(out=wt_sb[:, 11 * 128:20 * 128],
                                    in_=wts[:, 11 * 128:20 * 128])
            if r0 == 24:
                nc.sync.dma_start(out=wt_sb[:, 20 * 128:],
                                  in_=wts[:, 20 * 128:])
        bias_sb = const_pool.tile([128, MCH], F32)
        nc.sync.dma_start(out=bias_sb[:], in_=bias[:, :])

        # Warm-up matmuls: the PE clock ramps only after ~3.4us of
        # sustained activity, and the input DMA takes ~8us anyway, so
        # spend the wait ramping the clock on throwaway matmuls.
        def warm_group():
            wps = psi_pool.tile([128, NT], F32, tag="psi")
            for rep in range(2):
                nc.tensor.matmul(wps[:], lhsT=dummy[:, 0:128],
                                 rhs=dummy[:], start=(rep == 0),
                                 stop=(rep == 1))

        for _ in range(4):
            warm_group()

        # xhat: frequency-basis transform of the padded image. Only the
        # interior is computed (x pads are zero => xhat pads are zero);
        # the one-pixel border is zeroed once up front on idle engines.
        xhat = []
        for c in range(MCH):
            xh_t = xh_pool.tile([128, HP * HP], BF16, tag=f"xh{c}")
            xhat.append(xh_t)
        for c in range(MCH):
            # all on gpsimd: vector/scalar queues must stay clear for
            # the fwd drains that gate conv(0)
            e = nc.gpsimd
            e.memset(xhat[c][:, 0:HP], 0.0)                      # top row
            e.memset(xhat[c][:, (HP - 1) * HP:], 0.0)            # bottom row
            side = xhat[c][:].rearrange("p (h w) -> p h w", h=HP)
            e.memset(side[:, 1:HP - 1, 0:1], 0.0)                # left col
            e.memset(side[:, 1:HP - 1, HP - 1:HP], 0.0)          # right col

        # image-row ranges per fwd tile: 7 tiles of 8 rows
        fwd_rows = [(it * RPT, (it + 1) * RPT) for it in range(7)]

        def fwd_tile(it):
            """Transform image pixel rows [r0, r1)."""
            r0, r1 = fwd_rows[it]
            npx = (r1 - r0) * W
            ps = []
            for i in range(MCH):
                p = psf_pool.tile([128, NT], F32, tag="psf")
                nc.tensor.matmul(p[:, :npx], lhsT=blk(FWD_BLK(i)),
                                 rhs=xq[i][:, r0 * W:r1 * W],
                                 start=True, stop=True)
                ps.append(p)
            # psA = [xh0 lo | xh1 hi], psB = [xh1 lo | xh0 hi]; all four
            # copies are partition-aligned, written into the padded xhat
            # interior via 2D APs. psA drains on vector, psB on scalar
            # so the two PSUM banks drain in parallel, and the chunk0
            # halves drain first on each engine (conv does chunk0 first).
            dsts = [xh[:].rearrange("p (h w) -> p h w", h=HP)[
                        :, r0 + 1:r1 + 1, 1:1 + W] for xh in xhat]
            srcs = [p[:].rearrange("p (h w) -> p h w", w=W) for p in ps]
            nc.vector.tensor_copy(dsts[0][0:64], srcs[0][0:64])
            nc.vector.tensor_copy(dsts[1][64:128], srcs[0][64:128])
            nc.scalar.activation(dsts[0][64:128], srcs[1][64:128], ident)
            nc.scalar.activation(dsts[1][0:64], srcs[1][0:64], ident)

        # out viewed as [p(128), m(2), pix]: c = m*128 + p
        out_v = out.rearrange("(m p) h w -> p m (h w)", m=MCH)

        def conv_tile(nt):
            """Freq-domain conv for output tile nt -> assembled P, Q."""
            pscs = []
            for c in range(MCH):
                psum = psc_pool.tile([128, NT], F32, tag="psc")
                n_mm = 0
                xhv = xhat[c][:].rearrange("p (h w) -> p h w", h=HP)
                for kh in range(KK):
                    for kw in range(KK):
                        pos = kh * KK + kw
                        rhs = xhv[
                            :, nt * RPT + kh: nt * RPT + kh + RPT, kw: kw + W
                        ]
                        nc.tensor.matmul(
                            psum[:], lhsT=blk(CONV_BLK(pos, c)), rhs=rhs,
                            start=(n_mm == 0), stop=(n_mm == KK * KK - 1),
                        )
                        n_mm += 1
                pscs.append(psum)
            # P/Q feed inv passes for out chunk 0/1; partition-aligned
            # half-copies, vector on psc0's bank, scalar on psc1's. For
            # the last tile P completes first so inv m=0 starts sooner.
            P = oh_pool.tile([128, NT], BF16, tag="oh")
            Q = oh_pool.tile([128, NT], BF16, tag="oh")
            nc.vector.tensor_copy(P[0:64, :], pscs[0][0:64, :])
            nc.vector.tensor_copy(Q[64:128, :], pscs[0][64:128, :])
            if nt == N_TILES - 1:
                nc.scalar.activation(P[64:128, :], pscs[1][64:128, :], ident)
                nc.scalar.activation(Q[0:64, :], pscs[1][0:64, :], ident)
            else:
                nc.scalar.activation(Q[0:64, :], pscs[1][0:64, :], ident)
                nc.scalar.activation(P[64:128, :], pscs[1][64:128, :], ident)
            return [P, Q]

        def inv_tile(nt, pq, ob):
            """Inverse transform + bias for output tile nt, ship it."""
            last = nt == N_TILES - 1
            for m in range(MCH):
                # the last tile's psums come from the long-idle psf pool
                # so they don't wait on the psi rotation
                pool = psf_pool if last else psi_pool
                psum = pool.tile([128, NT], F32, tag="psf" if last else "psi")
                nc.tensor.matmul(psum[:], lhsT=blk(INV_BLK(m)), rhs=pq[m][:],
                                 start=True, stop=True)
                if m == 0:
                    nc.vector.tensor_scalar_add(ob[:, m, :], psum[:],
                                                bias_sb[:, m:m + 1])
                else:
                    nc.scalar.activation(ob[:, m, :], psum[:], ident,
                                         bias=bias_sb[:, m:m + 1])
                if nt >= N_TILES - 3:
                    # late tiles ship per-chunk so the final output
                    # transfers overlap the remaining compute; m1 of the
                    # last tile goes engine-locally from scalar, which
                    # also produced its bias-add
                    if last:
                        dma_eng = nc.sync if m == 0 else nc.scalar
                    else:
                        dma_eng = nc.gpsimd if m == 0 else nc.sync
                    dma_eng.dma_start(
                        out=out_v[:, m, ts(nt, NT)], in_=ob[:, m, :]
                    )
            if nt < N_TILES - 3:
                nc.gpsimd.dma_start(out=out_v[:, :, ts(nt, NT)], in_=ob[:])

        # Interleave: conv tile nt reads padded xhat rows [nt*8, nt*8+9]
        # = image rows [nt*8-1, nt*8+8] = fwd tiles nt-1..nt+1, so fwd
        # leads conv by two tiles (a deeper lead would pile fwd drains
        # onto vector/scalar while the PE idles); inv for tile nt is
        # issued after conv tile nt+1 so the P/Q copies complete in the
        # shadow of the next conv.
        fwd_tile(0)
        fwd_tile(1)
        # two more warm-up groups mask the fwd drain latency before
        # conv(0)'s matmuls can start
        warm_group()
        warm_group()
        pending = None
        for nt in range(N_TILES):
            pq = conv_tile(nt)
            if nt + 2 < len(fwd_rows):
                fwd_tile(nt + 2)
            if pending is not None:
                inv_tile(*pending)
            ob = ob_pool.tile([128, MCH, NT], BF16, tag="ob")
            pending = (nt, pq, ob)
        inv_tile(*pending)


def _build_nc():
    nc = bacc.Bacc("TRN2", target_bir_lowering=False, debug=False)
    x = nc.dram_tensor("x", [C, H, W], BF16, kind="ExternalInput").ap()
    wts = nc.dram_tensor("wts", [128, N_BLKS * 128], BF16,
                         kind="ExternalInput").ap()
    bias = nc.dram_tensor("bias", [128, MCH], F32, kind="ExternalInput").ap()
    out = nc.dram_tensor("out", [C, H, W], BF16, kind="ExternalOutput").ap()
    with tile.TileContext(nc) as tc:
        _kernel_body(tc, x, wts, bias, out)
    nc.compile()
    return nc


def kernel(x: np.ndarray, w: np.ndarray, b: np.ndarray) -> np.ndarray:
    global LAST_RESULT
    xp = np.ascontiguousarray(np.asarray(x, np.float32)).astype(NP_BF16)
    wts = _pack_weights(np.asarray(w, np.float32))
    b = np.ascontiguousarray(np.asarray(b, np.float32).reshape(MCH, 128).T)

    nc = _build_nc()
    in_maps = [{"x": xp[i], "wts": wts, "bias": b} for i in range(N_CORES)]
    trace = bool(int(os.environ.get("KERNEL_PROFILE", "0")))
    res = None
    last_err = None
    for attempt in range(3):
        try:
            res = run_bass_kernel_spmd(
                nc,
                in_maps,
                core_ids=list(range(N_CORES)),
                trace=trace,
            )
            break
        except Exception as e:  # transient device wedge -> retry
            last_err = e
    if res is None:
        raise last_err
    LAST_RESULT = res
    return np.stack(
        [res.results[i]["out"] for i in range(N_CORES)], axis=0
    ).astype(np.float32)


# revision 22
# speedup vs baseline: 1.0091x; 1.0091x over previous
"""BCM_Conv2d_fft kernel for Trainium2 (8 NeuronCores, batch-parallel).

The reference is a block-circulant 3x3 conv computed via per-block
rfft/irfft over the channel-block axis (block size 8). Per-frequency the
block products are independent, so in a real-DFT channel basis the
256->256 channel mixing matrix of each conv tap is block-diagonal with
frequency groups {f0:32, f4:32, f1:64} -> chunk0 and {f2:64, f3:64} ->
chunk1: the conv needs 9 matmuls per output tile per chunk (18 total),
which meets the K-streaming lower bound (9 positions x 256 components /
128 K-rows per pass).

The DFT (fwd) and inverse (inv) stages exploit a finer structure: each
channel block's components come only from its own 8 channels, so the
A / Ainv 128x128 chunk blocks have only 64 useful rows / cols. With a
partition layout that groups blocks 0-15 and 16-31 into 64-partition
halves (chunk1 flipped), fwd runs as 2 passes (one per input chunk,
producing halves of BOTH xhat chunks) and inv as 2 passes (one per
output chunk, consuming assembled P/Q tiles) - all inter-stage
PSUM->SBUF copies stay partition-aligned. Per output tile: 2 fwd + 18
conv + 2 inv passes (was 4 + 18 + 4).

Everything streams in bf16 (PSUM accumulates f32): same PE rate as
float32r at these tile sizes, but half the HBM traffic for x, weights
and out, and half-rate LDWEIGHTS via FWL. Max rel err ~3e-3 (tol 2e-2).

Sharding: batch B=8 -> one image per core.
"""

import os

import ml_dtypes
import numpy as np

import concourse.bacc as bacc
import concourse.mybir as mybir
import concourse.tile as tile
from concourse.bass import ts
from concourse.bass_utils import run_bass_kernel_spmd

N_CORES = 8
C = 256
H = W = 56
HP = H + 2
KK = 3
BS = 8
L = H * W
RPT = 8                  # output rows per tile
NT = RPT * W             # 448 pixels per tile
N_TILES = L // NT        # 7
MCH = C // 128           # 2 channel chunks

F32 = mybir.dt.float32
BF16 = mybir.dt.bfloat16
NP_BF16 = ml_dtypes.bfloat16

# weight block column indices in the packed wts tensor [128, 22*128]
FWD_BLK = lambda i: i                    # i = input chunk
CONV_BLK = lambda pos, c: 2 + pos * MCH + c
INV_BLK = lambda m: 20 + m               # m = output chunk
N_BLKS = 22

# real-DFT components per chunk: chunk0 = {f0, f1re, f1im, f4},
# chunk1 = {f2re, f2im, f3re, f3im} (closed under conv's re/im mixing)
C0 = [0, 1, 2, 7]
C1 = [3, 4, 5, 6]

LAST_RESULT = None


def _pc(c, bk, j):
    """Partition of (block bk, comp-index j) within xhat chunk c.

    chunk0: blocks 0-15 at parts 0-63; chunk1: blocks 16-31 at parts
    0-63 (flipped so all fwd/inv half-copies are partition-aligned).
    """
    if c == 0:
        return (bk % 16) * 4 + j + 64 * (bk // 16)
    return (bk % 16) * 4 + j + 64 * (1 - bk // 16)


def _pack_weights(w: np.ndarray) -> np.ndarray:
    """-> [128, 22*128] bf16: lhsT blocks for fwd(2), conv(18), inv(2)."""
    F = np.zeros((8, 8))
    FI = np.fft.rfft(np.eye(8), axis=-1)
    F[0] = FI[:, 0].real
    F[1], F[2] = FI[:, 1].real, FI[:, 1].imag
    F[3], F[4] = FI[:, 2].real, FI[:, 2].imag
    F[5], F[6] = FI[:, 3].real, FI[:, 3].imag
    F[7] = FI[:, 4].real
    Finv = np.linalg.inv(F)
    wf = np.fft.rfft(w.astype(np.float64), axis=-1)  # [32, 288, 5]

    wts = np.zeros((128, N_BLKS * 128), np.float64)

    def put(idx, lhsT):
        wts[:, idx * 128:(idx + 1) * 128] = lhsT

    # fwd pass i (K = x chunk i = blocks 16i..16i+15): M low half feeds
    # chunk i's parts 0-63, high half feeds the other chunk's parts
    # 64-127 (both hold blocks 16i..16i+15 by the _pc layout).
    for i in range(MCH):
        Lk = np.zeros((128, 128))
        own, other = (C0, C1) if i == 0 else (C1, C0)
        for bkl in range(16):
            for j, comp in enumerate(own):
                Lk[bkl * 8:(bkl + 1) * 8, bkl * 4 + j] = F[comp]
            for j, comp in enumerate(other):
                Lk[bkl * 8:(bkl + 1) * 8, 64 + bkl * 4 + j] = F[comp]
        put(FWD_BLK(i), Lk)

    for pos in range(9):
        for c in range(MCH):
            M = np.zeros((128, 128))
            for pb in range(32):
                rp = lambda j: _pc(c, pb, j)
                for kb in range(32):
                    cp = lambda j: _pc(c, kb, j)
                    Wc = wf[pb, pos * 32 + kb, :]
                    if c == 0:
                        M[rp(0), cp(0)] += Wc[0].real            # f0
                        M[rp(3), cp(3)] += Wc[4].real            # f4
                        Wr, Wi = Wc[1].real, Wc[1].imag          # f1
                        M[rp(1), cp(1)] += Wr
                        M[rp(1), cp(2)] += -Wi
                        M[rp(2), cp(1)] += Wi
                        M[rp(2), cp(2)] += Wr
                    else:
                        Wr, Wi = Wc[2].real, Wc[2].imag          # f2
                        M[rp(0), cp(0)] += Wr
                        M[rp(0), cp(1)] += -Wi
                        M[rp(1), cp(0)] += Wi
                        M[rp(1), cp(1)] += Wr
                        Wr, Wi = Wc[3].real, Wc[3].imag          # f3
                        M[rp(2), cp(2)] += Wr
                        M[rp(2), cp(3)] += -Wi
                        M[rp(3), cp(2)] += Wi
                        M[rp(3), cp(3)] += Wr
            put(CONV_BLK(pos, c), M.T)

    # inv pass m consumes P (m=0) / Q (m=1): parts 0-63 hold this out
    # chunk's blocks from its own-chunk conv psum, 64-127 from the other
    for mc in range(MCH):
        Lk = np.zeros((128, 128))
        for k in range(128):
            half, kk = k // 64, k % 64
            bkl, j = kk // 4, kk % 4
            comp = (C0 if (half == 0) == (mc == 0) else C1)[j]
            for e in range(8):
                Lk[k, bkl * 8 + e] = Finv[e, comp]
        put(INV_BLK(mc), Lk)
    return wts.astype(NP_BF16)


def _kernel_body(tc, x, wts, bias, out):
    nc = tc.nc
    ident = mybir.ActivationFunctionType.Identity
    with (
        tc.tile_pool(name="const", bufs=1) as const_pool,
        tc.tile_pool(name="xp", bufs=1) as xp_pool,
        tc.tile_pool(name="xh", bufs=1) as xh_pool,
        tc.tile_pool(name="oh", bufs=6) as oh_pool,
        tc.tile_pool(name="ob", bufs=4) as ob_pool,
        tc.tile_pool(name="psf", bufs=3, space="PSUM") as psf_pool,
        tc.tile_pool(name="psc", bufs=3, space="PSUM") as psc_pool,
        tc.tile_pool(name="psi", bufs=2, space="PSUM") as psi_pool,
    ):
        # DMA issue order is first-needed-first: fwd weights, then the
        # first input rows; conv weights stream on the ACT queue in
        # parallel with the remaining x rows on sync/gpsimd queues.
        wt_sb = const_pool.tile([128, N_BLKS * 128], BF16)
        blk = lambda idx: wt_sb[:, ts(idx, 128)]

        # dummy operand for PE warm-up matmuls (content irrelevant)
        dummy = const_pool.tile([128, NT], BF16)
        nc.gpsimd.memset(dummy[:], 0.0)

        # x arrives unpadded [C, H, W] and stays unpadded in SBUF so the
        # DMA moves large contiguous per-partition spans; the zero
        # padding is materialized in xhat by the fwd drain instead.
        xq = []
        for i in range(MCH):
            xq_t = xp_pool.tile([128, H * W], BF16, tag=f"xp{i}")
            xq.append(xq_t)
        row_splits = [0, 8, 16, 24, 32, 44, H]
        eng = [nc.sync, nc.gpsimd]
        for r0, r1 in zip(row_splits[:-1], row_splits[1:]):
            for i in range(MCH):
                eng[i].dma_start(
                    out=xq[i][:, r0 * W:r1 * W],
                    in_=x[ts(i, 128), r0:r1, :].rearrange("p h w -> p (h w)"),
                )
            if r0 == 0:
                # fwd weights lead the ACT queue so the first matmul is
                # gated only by the first x rows; conv weights follow
                nc.scalar.dma_start(out=wt_sb[:, 0:2 * 128],
                                    in_=wts[:, 0:2 * 128])
                nc.scalar.dma_start(out=wt_sb[:, 2 * 128:11 * 128],
                                    in_=wts[:, 2 * 128:11 * 128])
                nc.scalar.dma_start(out=wt_sb[:, 11 * 128:20 * 128],
                                    in_=wts[:, 11 * 128:20 * 128])
            if r0 == 24:
                nc.sync.dma_start(out=wt_sb[:, 20 * 128:],
                                  in_=wts[:, 20 * 128:])
        bias_sb = const_pool.tile([128, MCH], F32)
        nc.sync.dma_start(out=bias_sb[:], in_=bias[:, :])

        # Warm-up matmuls: the PE clock ramps only after ~3.4us of
        # sustained activity, and the input DMA takes ~8us anyway, so
        # spend the wait ramping the clock on throwaway matmuls.
        def warm_group():
            wps = psi_pool.tile([128, NT], F32, tag="psi")
            for rep in range(2):
                nc.tensor.matmul(wps[:], lhsT=dummy[:, 0:128],
                                 rhs=dummy[:], start=(rep == 0),
                                 stop=(rep == 1))

        for _ in range(4):
            warm_group()

        # xhat: frequency-basis transform of the padded image. Only the
        # interior is computed (x pads are zero => xhat pads are zero);
        # the one-pixel border is zeroed once up front on idle engines.
        xhat = []
        for c in range(MCH):
            xh_t = xh_pool.tile([128, HP * HP], BF16, tag=f"xh{c}")
            xhat.append(xh_t)
        for c in range(MCH):
            e = [nc.gpsimd, nc.vector][c]
            e.memset(xhat[c][:, 0:HP], 0.0)                      # top row
            e.memset(xhat[c][:, (HP - 1) * HP:], 0.0)            # bottom row
            side = xhat[c][:].rearrange("p (h w) -> p h w", h=HP)
            e.memset(side[:, 1:HP - 1, 0:1], 0.0)                # left col
            e.memset(side[:, 1:HP - 1, HP - 1:HP], 0.0)          # right col

        # image-row ranges per fwd tile: 7 tiles of 8 rows
        fwd_rows = [(it * RPT, (it + 1) * RPT) for it in range(7)]

        def fwd_tile(it):
            """Transform image pixel rows [r0, r1)."""
            r0, r1 = fwd_rows[it]
            npx = (r1 - r0) * W
            ps = []
            for i in range(MCH):
                p = psf_pool.tile([128, NT], F32, tag="psf")
                nc.tensor.matmul(p[:, :npx], lhsT=blk(FWD_BLK(i)),
                                 rhs=xq[i][:, r0 * W:r1 * W],
                                 start=True, stop=True)
                ps.append(p)
            # psA = [xh0 lo | xh1 hi], psB = [xh1 lo | xh0 hi]; all four
            # copies are partition-aligned, written into the padded xhat
            # interior via 2D APs. psA drains on vector, psB on scalar
            # so the two PSUM banks drain in parallel, and the chunk0
            # halves drain first on each engine (conv does chunk0 first).
            dsts = [xh[:].rearrange("p (h w) -> p h w", h=HP)[
                        :, r0 + 1:r1 + 1, 1:1 + W] for xh in xhat]
            srcs = [p[:].rearrange("p (h w) -> p h w", w=W) for p in ps]
            nc.vector.tensor_copy(dsts[0][0:64], srcs[0][0:64])
            nc.vector.tensor_copy(dsts[1][64:128], srcs[0][64:128])
            nc.scalar.activation(dsts[0][64:128], srcs[1][64:128], ident)
            nc.scalar.activation(dsts[1][0:64], srcs[1][0:64], ident)

        # out viewed as [p(128), m(2), pix]: c = m*128 + p
        out_v = out.rearrange("(m p) h w -> p m (h w)", m=MCH)

        def conv_tile(nt):
            """Freq-domain conv for output tile nt -> assembled P, Q."""
            pscs = []
            for c in range(MCH):
                psum = psc_pool.tile([128, NT], F32, tag="psc")
                n_mm = 0
                xhv = xhat[c][:].rearrange("p (h w) -> p h w", h=HP)
                for kh in range(KK):
                    for kw in range(KK):
                        pos = kh * KK + kw
                        rhs = xhv[
                            :, nt * RPT + kh: nt * RPT + kh + RPT, kw: kw + W
                        ]
                        nc.tensor.matmul(
                            psum[:], lhsT=blk(CONV_BLK(pos, c)), rhs=rhs,
                            start=(n_mm == 0), stop=(n_mm == KK * KK - 1),
                        )
                        n_mm += 1
                pscs.append(psum)
            # P/Q feed inv passes for out chunk 0/1; partition-aligned
            # half-copies, vector on psc0's bank, scalar on psc1's. For
            # the last tile P completes first so inv m=0 starts sooner.
            P = oh_pool.tile([128, NT], BF16, tag="oh")
            Q = oh_pool.tile([128, NT], BF16, tag="oh")
            nc.vector.tensor_copy(P[0:64, :], pscs[0][0:64, :])
            nc.vector.tensor_copy(Q[64:128, :], pscs[0][64:128, :])
            if nt == N_TILES - 1:
                nc.scalar.activation(P[64:128, :], pscs[1][64:128, :], ident)
                nc.scalar.activation(Q[0:64, :], pscs[1][0:64, :], ident)
            else:
                nc.scalar.activation(Q[0:64, :], pscs[1][0:64, :], ident)
                nc.scalar.activation(P[64:128, :], pscs[1][64:128, :], ident)
            return [P, Q]

        def inv_tile(nt, pq, ob):
            """Inverse transform + bias for output tile nt, ship it."""
            last = nt == N_TILES - 1
            for m in range(MCH):
                # the last tile's psums come from the long-idle psf pool
                # so they don't wait on the psi rotation
                pool = psf_pool if last else psi_pool
                psum = pool.tile([128, NT], F32, tag="psf" if last else "psi")
                nc.tensor.matmul(psum[:], lhsT=blk(INV_BLK(m)), rhs=pq[m][:],
                                 start=True, stop=True)
                if m == 0:
                    nc.vector.tensor_scalar_add(ob[:, m, :], psum[:],
                                                bias_sb[:, m:m + 1])
                else:
                    nc.scalar.activation(ob[:, m, :], psum[:], ident,
                                         bias=bias_sb[:, m:m + 1])
                if nt >= N_TILES - 3:
                    # late tiles ship per-chunk so the final output
                    # transfers overlap the remaining compute; m1 of the
                    # last tile goes engine-locally from scalar, which
                    # also produced its bias-add
                    if last:
                        dma_eng = nc.sync if m == 0 else nc.scalar
                    else:
                        dma_eng = nc.gpsimd if m == 0 else nc.sync
                    dma_eng.dma_start(
                        out=out_v[:, m, ts(nt, NT)], in_=ob[:, m, :]
                    )
            if nt < N_TILES - 3:
                nc.gpsimd.dma_start(out=out_v[:, :, ts(nt, NT)], in_=ob[:])

        # Interleave: conv tile nt reads padded xhat rows [nt*8, nt*8+9]
        # = image rows [nt*8-1, nt*8+8] = fwd tiles nt-1..nt+1, so fwd
        # leads conv by two tiles (a deeper lead would pile fwd drains
        # onto vector/scalar while the PE idles); inv for tile nt is
        # issued after conv tile nt+1 so the P/Q copies complete in the
        # shadow of the next conv.
        fwd_tile(0)
        fwd_tile(1)
        # two more warm-up groups mask the fwd drain latency before
        # conv(0)'s matmuls can start
        warm_group()
        warm_group()
        pending = None
        for nt in range(N_TILES):
            pq = conv_tile(nt)
            if nt + 2 < len(fwd_rows):
                fwd_tile(nt + 2)
            if pending is not None:
                inv_tile(*pending)
            ob = ob_pool.tile([128, MCH, NT], BF16, tag="ob")
            pending = (nt, pq, ob)
        inv_tile(*pending)


def _build_nc():
    nc = bacc.Bacc("TRN2", target_bir_lowering=False, debug=False)
    x = nc.dram_tensor("x", [C, H, W], BF16, kind="ExternalInput").ap()
    wts = nc.dram_tensor("wts", [128, N_BLKS * 128], BF16,
                         kind="ExternalInput").ap()
    bias = nc.dram_tensor("bias", [128, MCH], F32, kind="ExternalInput").ap()
    out = nc.dram_tensor("out", [C, H, W], BF16, kind="ExternalOutput").ap()
    with tile.TileContext(nc) as tc:
        _kernel_body(tc, x, wts, bias, out)
    nc.compile()
    return nc


def kernel(x: np.ndarray, w: np.ndarray, b: np.ndarray) -> np.ndarray:
    global LAST_RESULT
    xp = np.ascontiguousarray(np.asarray(x, np.float32)).astype(NP_BF16)
    wts = _pack_weights(np.asarray(w, np.float32))
    b = np.ascontiguousarray(np.asarray(b, np.float32).reshape(MCH, 128).T)

    nc = _build_nc()
    in_maps = [{"x": xp[i], "wts": wts, "bias": b} for i in range(N_CORES)]
    trace = bool(int(os.environ.get("KERNEL_PROFILE", "0")))
    res = None
    last_err = None
    for attempt in range(3):
        try:
            res = run_bass_kernel_spmd(
                nc,
                in_maps,
                core_ids=list(range(N_CORES)),
                trace=trace,
            )
            break
        except Exception as e:  # transient device wedge -> retry
            last_err = e
    if res is None:
        raise last_err
    LAST_RESULT = res
    return np.stack(
        [res.results[i]["out"] for i in range(N_CORES)], axis=0
    ).astype(np.float32)


# revision 23
# speedup vs baseline: 1.1334x; 1.1232x over previous
"""BCM_Conv2d_fft kernel for Trainium2 (8 NeuronCores, batch-parallel).

The reference is a block-circulant 3x3 conv computed via per-block
rfft/irfft over the channel-block axis (block size 8). Per-frequency the
block products are independent, so in a real-DFT channel basis the
256->256 channel mixing matrix of each conv tap is block-diagonal with
frequency groups {f0:32, f4:32, f1:64} -> chunk0 and {f2:64, f3:64} ->
chunk1: the conv needs 9 matmuls per output tile per chunk (18 total),
which meets the K-streaming lower bound (9 positions x 256 components /
128 K-rows per pass).

The DFT (fwd) and inverse (inv) stages exploit a finer structure: each
channel block's components come only from its own 8 channels, so the
A / Ainv 128x128 chunk blocks have only 64 useful rows / cols. With a
partition layout that groups blocks 0-15 and 16-31 into 64-partition
halves (chunk1 flipped), fwd runs as 2 passes (one per input chunk,
producing halves of BOTH xhat chunks) and inv as 2 passes (one per
output chunk, consuming assembled P/Q tiles) - all inter-stage
PSUM->SBUF copies stay partition-aligned. Per output tile: 2 fwd + 18
conv + 2 inv passes (was 4 + 18 + 4).

Everything streams in bf16 (PSUM accumulates f32): same PE rate as
float32r at these tile sizes, but half the HBM traffic for x, weights
and out, and half-rate LDWEIGHTS via FWL. Max rel err ~3e-3 (tol 2e-2).

Sharding: batch B=8 -> one image per core.
"""

import os

import ml_dtypes
import numpy as np

import concourse.bacc as bacc
import concourse.mybir as mybir
import concourse.tile as tile
from concourse.bass import ts
from concourse.bass_utils import run_bass_kernel_spmd

N_CORES = 8
C = 256
H = W = 56
HP = H + 2
KK = 3
BS = 8
L = H * W
RPT = 8                  # output rows per tile
NT = RPT * W             # 448 pixels per tile
N_TILES = L // NT        # 7
MCH = C // 128           # 2 channel chunks

F32 = mybir.dt.float32
BF16 = mybir.dt.bfloat16
NP_BF16 = ml_dtypes.bfloat16

# weight block column indices in the packed wts tensor [128, 22*128]
FWD_BLK = lambda i: i                    # i = input chunk
CONV_BLK = lambda pos, c: 2 + pos * MCH + c
INV_BLK = lambda m: 20 + m               # m = output chunk
N_BLKS = 22

# real-DFT components per chunk: chunk0 = {f0, f1re, f1im, f4},
# chunk1 = {f2re, f2im, f3re, f3im} (closed under conv's re/im mixing)
C0 = [0, 1, 2, 7]
C1 = [3, 4, 5, 6]

LAST_RESULT = None


def _pc(c, bk, j):
    """Partition of (block bk, comp-index j) within xhat chunk c.

    chunk0: blocks 0-15 at parts 0-63; chunk1: blocks 16-31 at parts
    0-63 (flipped so all fwd/inv half-copies are partition-aligned).
    """
    if c == 0:
        return (bk % 16) * 4 + j + 64 * (bk // 16)
    return (bk % 16) * 4 + j + 64 * (1 - bk // 16)


def _pack_weights(w: np.ndarray) -> np.ndarray:
    """-> [128, 22*128] bf16: lhsT blocks for fwd(2), conv(18), inv(2)."""
    F = np.zeros((8, 8))
    FI = np.fft.rfft(np.eye(8), axis=-1)
    F[0] = FI[:, 0].real
    F[1], F[2] = FI[:, 1].real, FI[:, 1].imag
    F[3], F[4] = FI[:, 2].real, FI[:, 2].imag
    F[5], F[6] = FI[:, 3].real, FI[:, 3].imag
    F[7] = FI[:, 4].real
    Finv = np.linalg.inv(F)
    wf = np.fft.rfft(w.astype(np.float64), axis=-1)  # [32, 288, 5]

    wts = np.zeros((128, N_BLKS * 128), np.float64)

    def put(idx, lhsT):
        wts[:, idx * 128:(idx + 1) * 128] = lhsT

    # fwd pass i (K = x chunk i = blocks 16i..16i+15): M low half feeds
    # chunk i's parts 0-63, high half feeds the other chunk's parts
    # 64-127 (both hold blocks 16i..16i+15 by the _pc layout).
    for i in range(MCH):
        Lk = np.zeros((128, 128))
        own, other = (C0, C1) if i == 0 else (C1, C0)
        for bkl in range(16):
            for j, comp in enumerate(own):
                Lk[bkl * 8:(bkl + 1) * 8, bkl * 4 + j] = F[comp]
            for j, comp in enumerate(other):
                Lk[bkl * 8:(bkl + 1) * 8, 64 + bkl * 4 + j] = F[comp]
        put(FWD_BLK(i), Lk)

    for pos in range(9):
        for c in range(MCH):
            M = np.zeros((128, 128))
            for pb in range(32):
                rp = lambda j: _pc(c, pb, j)
                for kb in range(32):
                    cp = lambda j: _pc(c, kb, j)
                    Wc = wf[pb, pos * 32 + kb, :]
                    if c == 0:
                        M[rp(0), cp(0)] += Wc[0].real            # f0
                        M[rp(3), cp(3)] += Wc[4].real            # f4
                        Wr, Wi = Wc[1].real, Wc[1].imag          # f1
                        M[rp(1), cp(1)] += Wr
                        M[rp(1), cp(2)] += -Wi
                        M[rp(2), cp(1)] += Wi
                        M[rp(2), cp(2)] += Wr
                    else:
                        Wr, Wi = Wc[2].real, Wc[2].imag          # f2
                        M[rp(0), cp(0)] += Wr
                        M[rp(0), cp(1)] += -Wi
                        M[rp(1), cp(0)] += Wi
                        M[rp(1), cp(1)] += Wr
                        Wr, Wi = Wc[3].real, Wc[3].imag          # f3
                        M[rp(2), cp(2)] += Wr
                        M[rp(2), cp(3)] += -Wi
                        M[rp(3), cp(2)] += Wi
                        M[rp(3), cp(3)] += Wr
            put(CONV_BLK(pos, c), M.T)

    # inv pass m consumes P (m=0) / Q (m=1): parts 0-63 hold this out
    # chunk's blocks from its own-chunk conv psum, 64-127 from the other
    for mc in range(MCH):
        Lk = np.zeros((128, 128))
        for k in range(128):
            half, kk = k // 64, k % 64
            bkl, j = kk // 4, kk % 4
            comp = (C0 if (half == 0) == (mc == 0) else C1)[j]
            for e in range(8):
                Lk[k, bkl * 8 + e] = Finv[e, comp]
        put(INV_BLK(mc), Lk)
    return wts.astype(NP_BF16)


def _kernel_body(tc, x, wts, bias, out):
    nc = tc.nc
    ident = mybir.ActivationFunctionType.Identity
    with (
        tc.tile_pool(name="const", bufs=1) as const_pool,
        tc.tile_pool(name="xp", bufs=1) as xp_pool,
        tc.tile_pool(name="xh", bufs=1) as xh_pool,
        tc.tile_pool(name="oh", bufs=6) as oh_pool,
        tc.tile_pool(name="ob", bufs=4) as ob_pool,
        tc.tile_pool(name="psf", bufs=3, space="PSUM") as psf_pool,
        tc.tile_pool(name="psc", bufs=3, space="PSUM") as psc_pool,
        tc.tile_pool(name="psi", bufs=2, space="PSUM") as psi_pool,
    ):
        # DMA issue order is first-needed-first: fwd weights, then the
        # first input rows; conv weights stream on the ACT queue in
        # parallel with the remaining x rows on sync/gpsimd queues.
        wt_sb = const_pool.tile([128, N_BLKS * 128], BF16)
        blk = lambda idx: wt_sb[:, ts(idx, 128)]

        # dummy operand for PE warm-up matmuls (content irrelevant)
        dummy = const_pool.tile([128, NT], BF16)
        nc.gpsimd.memset(dummy[:], 0.0)

        # x arrives unpadded [C, H, W] and stays unpadded in SBUF so the
        # DMA moves large contiguous per-partition spans; the zero
        # padding is materialized in xhat by the fwd drain instead.
        xq = []
        for i in range(MCH):
            xq_t = xp_pool.tile([128, H * W], BF16, tag=f"xp{i}")
            xq.append(xq_t)
        row_splits = [0, 8, 16, 24, 32, 44, H]
        eng = [nc.sync, nc.gpsimd]
        for r0, r1 in zip(row_splits[:-1], row_splits[1:]):
            for i in range(MCH):
                eng[i].dma_start(
                    out=xq[i][:, r0 * W:r1 * W],
                    in_=x[ts(i, 128), r0:r1, :].rearrange("p h w -> p (h w)"),
                )
            if r0 == 0:
                # fwd weights lead the ACT queue so the first matmul is
                # gated only by the first x rows; conv weights follow
                nc.scalar.dma_start(out=wt_sb[:, 0:2 * 128],
                                    in_=wts[:, 0:2 * 128])
                nc.scalar.dma_start(out=wt_sb[:, 2 * 128:11 * 128],
                                    in_=wts[:, 2 * 128:11 * 128])
                nc.scalar.dma_start(out=wt_sb[:, 11 * 128:20 * 128],
                                    in_=wts[:, 11 * 128:20 * 128])
            if r0 == 24:
                nc.sync.dma_start(out=wt_sb[:, 20 * 128:],
                                  in_=wts[:, 20 * 128:])
        bias_sb = const_pool.tile([128, MCH], F32)
        nc.sync.dma_start(out=bias_sb[:], in_=bias[:, :])

        # Warm-up matmuls: the PE clock ramps only after ~3.4us of
        # sustained activity, and the input DMA takes ~8us anyway, so
        # spend the wait ramping the clock on throwaway matmuls.
        def warm_group():
            wps = psi_pool.tile([128, NT], F32, tag="psi")
            for rep in range(2):
                nc.tensor.matmul(wps[:], lhsT=dummy[:, 0:128],
                                 rhs=dummy[:], start=(rep == 0),
                                 stop=(rep == 1))

        for _ in range(4):
            warm_group()

        # xhat: frequency-basis transform of the padded image. Only the
        # interior is computed (x pads are zero => xhat pads are zero);
        # the one-pixel border is zeroed once up front on idle engines.
        xhat = []
        for c in range(MCH):
            xh_t = xh_pool.tile([128, HP * HP], BF16, tag=f"xh{c}")
            xhat.append(xh_t)
        for c in range(MCH):
            e = [nc.gpsimd, nc.vector][c]
            e.memset(xhat[c][:, 0:HP], 0.0)                      # top row
            e.memset(xhat[c][:, (HP - 1) * HP:], 0.0)            # bottom row
            side = xhat[c][:].rearrange("p (h w) -> p h w", h=HP)
            e.memset(side[:, 1:HP - 1, 0:1], 0.0)                # left col
            e.memset(side[:, 1:HP - 1, HP - 1:HP], 0.0)          # right col

        # image-row ranges per fwd tile: 7 tiles of 8 rows
        fwd_rows = [(it * RPT, (it + 1) * RPT) for it in range(7)]

        def fwd_tile(it):
            """Transform image pixel rows [r0, r1)."""
            r0, r1 = fwd_rows[it]
            npx = (r1 - r0) * W
            ps = []
            for i in range(MCH):
                p = psf_pool.tile([128, NT], F32, tag="psf")
                nc.tensor.matmul(p[:, :npx], lhsT=blk(FWD_BLK(i)),
                                 rhs=xq[i][:, r0 * W:r1 * W],
                                 start=True, stop=True)
                ps.append(p)
            # psA = [xh0 lo | xh1 hi], psB = [xh1 lo | xh0 hi]; all four
            # copies are partition-aligned, written into the padded xhat
            # interior via 2D APs. psA drains on vector, psB on scalar
            # so the two PSUM banks drain in parallel, and the chunk0
            # halves drain first on each engine (conv does chunk0 first).
            dsts = [xh[:].rearrange("p (h w) -> p h w", h=HP)[
                        :, r0 + 1:r1 + 1, 1:1 + W] for xh in xhat]
            srcs = [p[:].rearrange("p (h w) -> p h w", w=W) for p in ps]
            nc.vector.tensor_copy(dsts[0][0:64], srcs[0][0:64])
            nc.vector.tensor_copy(dsts[1][64:128], srcs[0][64:128])
            nc.scalar.activation(dsts[0][64:128], srcs[1][64:128], ident)
            nc.scalar.activation(dsts[1][0:64], srcs[1][0:64], ident)

        # out viewed as [p(128), m(2), pix]: c = m*128 + p
        out_v = out.rearrange("(m p) h w -> p m (h w)", m=MCH)

        def conv_tile(nt):
            """Freq-domain conv for output tile nt -> assembled P, Q."""
            pscs = []
            for c in range(MCH):
                psum = psc_pool.tile([128, NT], F32, tag="psc")
                n_mm = 0
                xhv = xhat[c][:].rearrange("p (h w) -> p h w", h=HP)
                for kh in range(KK):
                    for kw in range(KK):
                        pos = kh * KK + kw
                        rhs = xhv[
                            :, nt * RPT + kh: nt * RPT + kh + RPT, kw: kw + W
                        ]
                        nc.tensor.matmul(
                            psum[:], lhsT=blk(CONV_BLK(pos, c)), rhs=rhs,
                            start=(n_mm == 0), stop=(n_mm == KK * KK - 1),
                        )
                        n_mm += 1
                pscs.append(psum)
            # P/Q feed inv passes for out chunk 0/1; partition-aligned
            # half-copies, vector on psc0's bank, scalar on psc1's. For
            # the last tile P completes first so inv m=0 starts sooner.
            P = oh_pool.tile([128, NT], BF16, tag="oh")
            Q = oh_pool.tile([128, NT], BF16, tag="oh")
            nc.vector.tensor_copy(P[0:64, :], pscs[0][0:64, :])
            nc.vector.tensor_copy(Q[64:128, :], pscs[0][64:128, :])
            if nt == N_TILES - 1:
                nc.scalar.activation(P[64:128, :], pscs[1][64:128, :], ident)
                nc.scalar.activation(Q[0:64, :], pscs[1][0:64, :], ident)
            else:
                nc.scalar.activation(Q[0:64, :], pscs[1][0:64, :], ident)
                nc.scalar.activation(P[64:128, :], pscs[1][64:128, :], ident)
            return [P, Q]

        def inv_tile(nt, pq, ob):
            """Inverse transform + bias for output tile nt, ship it."""
            last = nt == N_TILES - 1
            for m in range(MCH):
                # the last tile's psums come from the long-idle psf pool
                # so they don't wait on the psi rotation
                pool = psf_pool if last else psi_pool
                psum = pool.tile([128, NT], F32, tag="psf" if last else "psi")
                nc.tensor.matmul(psum[:], lhsT=blk(INV_BLK(m)), rhs=pq[m][:],
                                 start=True, stop=True)
                if m == 0:
                    nc.vector.tensor_scalar_add(ob[:, m, :], psum[:],
                                                bias_sb[:, m:m + 1])
                else:
                    nc.scalar.activation(ob[:, m, :], psum[:], ident,
                                         bias=bias_sb[:, m:m + 1])
                if last:
                    # m1 (the later chain) ships engine-locally from
                    # scalar, which also produced its bias-add
                    dma_eng = nc.sync if m == 0 else nc.scalar
                    dma_eng.dma_start(
                        out=out_v[:, m, ts(nt, NT)], in_=ob[:, m, :]
                    )
            if not last:
                nc.gpsimd.dma_start(out=out_v[:, :, ts(nt, NT)], in_=ob[:])

        # Interleave: conv tile nt reads padded xhat rows [nt*8, nt*8+9]
        # = image rows [nt*8-1, nt*8+8] = fwd tiles nt-1..nt+1, so fwd
        # leads conv by two tiles (a deeper lead would pile fwd drains
        # onto vector/scalar while the PE idles); inv for tile nt is
        # issued after conv tile nt+1 so the P/Q copies complete in the
        # shadow of the next conv.
        fwd_tile(0)
        fwd_tile(1)
        # two more warm-up groups mask the fwd drain latency before
        # conv(0)'s matmuls can start
        warm_group()
        warm_group()
        pending = None
        for nt in range(N_TILES):
            pq = conv_tile(nt)
            if nt + 2 < len(fwd_rows):
                fwd_tile(nt + 2)
            if pending is not None:
                inv_tile(*pending)
            ob = ob_pool.tile([128, MCH, NT], BF16, tag="ob")
            pending = (nt, pq, ob)
        inv_tile(*pending)


def _build_nc():
    nc = bacc.Bacc("TRN2", target_bir_lowering=False, debug=False)
    x = nc.dram_tensor("x", [C, H, W], BF16, kind="ExternalInput").ap()
    wts = nc.dram_tensor("wts", [128, N_BLKS * 128], BF16,
                         kind="ExternalInput").ap()
    bias = nc.dram_tensor("bias", [128, MCH], F32, kind="ExternalInput").ap()
    out = nc.dram_tensor("out", [C, H, W], BF16, kind="ExternalOutput").ap()
    with tile.TileContext(nc) as tc:
        _kernel_body(tc, x, wts, bias, out)
    nc.compile()
    return nc


def kernel(x: np.ndarray, w: np.ndarray, b: np.ndarray) -> np.ndarray:
    global LAST_RESULT
    xp = np.ascontiguousarray(np.asarray(x, np.float32)).astype(NP_BF16)
    wts = _pack_weights(np.asarray(w, np.float32))
    b = np.ascontiguousarray(np.asarray(b, np.float32).reshape(MCH, 128).T)

    nc = _build_nc()
    in_maps = [{"x": xp[i], "wts": wts, "bias": b} for i in range(N_CORES)]
    trace = bool(int(os.environ.get("KERNEL_PROFILE", "0")))
    res = None
    last_err = None
    for attempt in range(3):
        try:
            res = run_bass_kernel_spmd(
                nc,
                in_maps,
                core_ids=list(range(N_CORES)),
                trace=trace,
            )
            break
        except Exception as e:  # transient device wedge -> retry
            last_err = e
    if res is None:
        raise last_err
    LAST_RESULT = res
    return np.stack(
        [res.results[i]["out"] for i in range(N_CORES)], axis=0
    ).astype(np.float32)
